# revision 4
# baseline (speedup 1.0000x reference)
"""Trainium2 Bass kernel for per-sample covariance pooling + FC + L2 normalize.

Reference computation (per sample of x [B=32, N=50000, D=64]):
    xc  = x - mean(x, axis=N)
    cov = xc^T xc / (N-1)               # [64, 64]
    out = cov.flatten() @ W.T + b       # [256]
    out = out / max(||out||_2, 1e-12)

Sharding: data-parallel over batch B across 8 NeuronCores (4 samples/core).
W (fed pre-transposed and pre-tiled) and b (pre-broadcast) are replicated.

Host-side marshalling appends a ones column to x and zero-pads rows to a
whole number of 128-row n-tiles (-> [B, 50048, 65]); zero rows are inert
for both reductions.  The ones column lets a single accumulating matmul
per [128, 65] tile produce both S = X^T X (PSUM rows 0:64) and the column
sums s (row 64), with every DMA fully contiguous on both sides.

Per-core algorithm:
  - x streams through THREE DMA queues in parallel: the SWDGE queue
    (gpsimd) with an inline fp32->bf16 cast, plus the SP and Activation
    HWDGE rings into fp32 staging tiles that the (otherwise idle) DVE
    casts to bf16.  A single queue measured ~430 GB/s; the queues share
    the 16 DMA engines but not a queue bottleneck.
  - 391 accumulating PE matmuls per sample (bf16, K=128, M=65, N=64);
    the final chunk of the last sample is split into pieces so only the
    last piece's matmuls trail the final DMA byte.
  - Mean correction: scale s on partition 64, then a K=1 outer-product
    matmul accumulates -(s/sqrt(N))(s/sqrt(N))^T into PSUM rows 0:64.
    The ScalarE sqrt LUT is pre-warmed from a memset tile (NO DMA
    dependency — a DMA-fed warm op once stalled PE 34us mid-stream).
  - cov -> cov2[128, s, 32] fp16 with partition p = e + 64*(t%2): the
    even-t half is an ACT copy, the odd-t half a 4KiB cross-partition
    SBUF->SBUF DMA on the Act ring.  FC then contracts K=128 in 32
    matmuls (vs 64 at K=64) — the FC tail runs inside a hardware
    throttle window, so halving its matmul count halves the tail.
  - bias add, L2 normalize (DVE/ACT), DMA out [4, 256] per core.
"""

import math
import numpy as np
from contextlib import ExitStack

import concourse.bass as bass
import concourse.tile as tile
from concourse import bacc, mybir
from concourse import bass_utils
from concourse._compat import with_exitstack

B, N_FULL, D, OUT = 32, 50000, 64, 256
DA = D + 1  # x augmented with a ones column
NCORES = 8
BPC = B // NCORES  # samples per core
P = 128  # partitions per n-tile
NT = (N_FULL + P - 1) // P  # 391 n-tiles per sample (pad 50000 -> 50048)
N_ROWS = NT * P  # 50048
CHUNK_T = 56

# Chunk schedule (in n-tiles) and queue routing.  'sw' = SWDGE cast
# stream, 'sp'/'act' = fp32 over the SP/Act HWDGE ring + DVE cast.
CHUNKS = [(56, "sw"), (56, "sp"), (56, "sw"), (56, "act"), (56, "sw"),
          (56, "sp"), (55, "sw")]
CHUNKS_LAST = [(56, "sw"), (56, "sp"), (56, "act"), (56, "sw"), (56, "sp"),
               (56, "sw"), (28, "sw"), (14, "sw"), (7, "sw"), (6, "sw")]
assert sum(t for t, _ in CHUNKS) == NT and sum(t for t, _ in CHUNKS_LAST) == NT

F32 = mybir.dt.float32
BF16 = mybir.dt.bfloat16
FC_DT = mybir.dt.float16  # FC runs at bf16 speed with 2^-11 rounding


@with_exitstack
def _cov_kernel(
    ctx: ExitStack,
    tc: tile.TileContext,
    out: bass.AP,
    xs: bass.AP,
    wt: bass.AP,
    b4: bass.AP,
    n_true: int,
):
    nc = tc.nc
    inv_sqrt_n = 1.0 / math.sqrt(n_true)
    inv_nm1 = 1.0 / (n_true - 1)

    xsf = xs.rearrange("b n e -> (b n) e")  # [BPC*N_ROWS, 65]

    chunks = ctx.enter_context(tc.tile_pool(name="chunks", bufs=8))
    stage_p = ctx.enter_context(tc.tile_pool(name="stage_p", bufs=2))
    stage_a = ctx.enter_context(tc.tile_pool(name="stage_a", bufs=2))
    smalls = ctx.enter_context(tc.tile_pool(name="smalls", bufs=4))
    singles = ctx.enter_context(tc.tile_pool(name="singles", bufs=1))
    psum_s = ctx.enter_context(tc.tile_pool(name="psum_s", bufs=2, space="PSUM"))
    psum_fc = ctx.enter_context(tc.tile_pool(name="psum_fc", bufs=2, space="PSUM"))

    # Replicated FC weights on the SP HWDGE ring: bias first (tiny), then
    # W^T pre-tiled host-side to [p=e+64w, u, o] (t = 2u+w) so each
    # partition is ONE contiguous 16KiB descriptor.
    b4_sb = singles.tile([BPC, OUT], F32)
    nc.sync.dma_start(out=b4_sb, in_=b4)
    wt_sb = singles.tile([128, 32, OUT], FC_DT)
    nc.sync.dma_start(out=wt_sb, in_=wt.rearrange("p (u o) -> p u o", o=OUT))

    # cov2[e + 64w, s, u] = cov_s[t=2u+w, :] column slices (cov symmetric)
    cov2 = singles.tile([128, BPC, 32], FC_DT)

    # Preload the ScalarE Sqrt LUT from a memset tile: NO DMA dependency.
    warm = singles.tile([1, 2], F32)
    nc.vector.memset(warm[:, 0:1], 1.0)
    nc.scalar.sqrt(warm[:, 1:2], warm[:, 0:1])

    for s in range(BPC):
        ps = psum_s.tile([65, 64], F32)
        chunk_list = CHUNKS_LAST if s == BPC - 1 else CHUNKS
        r0 = s * N_ROWS
        n_tiles_done = 0
        for tcnt, route in chunk_list:
            src = xsf[r0 : r0 + tcnt * P, :].rearrange("(p q) e -> p q e", q=tcnt)
            r0 += tcnt * P
            ctile = chunks.tile([P, tcnt, DA], BF16, tag="ctile")
            if route == "sw":
                nc.gpsimd.dma_start(out=ctile, in_=src)
            else:
                pool, eng = (
                    (stage_p, nc.sync) if route == "sp" else (stage_a, nc.scalar)
                )
                stg = pool.tile([P, tcnt, DA], F32, tag="stage")
                eng.dma_start(out=stg, in_=src)
                nc.vector.tensor_scalar_mul(ctile, stg, 1.0)
            for q in range(tcnt):
                nc.tensor.matmul(
                    ps,
                    lhsT=ctile[:, q, :],
                    rhs=ctile[:, q, 0:64],
                    start=(n_tiles_done == 0),
                    stop=(n_tiles_done == NT - 1),
                )
                n_tiles_done += 1

        # Column sums s sit in PSUM row 64.  Scale into SBUF on the same
        # partition; the K=1 outer-product matmul runs from partition 64,
        # accumulating -s s^T / N into rows 0:64.
        sboth = smalls.tile([65, 2, 64], F32)
        nc.scalar.mul(sboth[64:65, 0, :], ps[64:65, :], inv_sqrt_n)
        nc.scalar.mul(sboth[64:65, 1, :], ps[64:65, :], -inv_sqrt_n)
        nc.tensor.matmul(
            ps[0:64, :],
            lhsT=sboth[64:65, 0, :],
            rhs=sboth[64:65, 1, :],
            start=False,
            stop=True,
            skip_group_check=True,
        )
        # cov2 fill: even t on partitions 0:64 (ACT copy), odd t on
        # partitions 64:128 (cross-partition SBUF->SBUF DMA, Act ring).
        ps2 = ps[0:64, :].rearrange("p (u w) -> p w u", w=2)
        nc.scalar.mul(out=cov2[0:64, s, :], in_=ps2[:, 0, :], mul=inv_nm1)
        odd = smalls.tile([64, 32], FC_DT)
        nc.scalar.mul(out=odd, in_=ps2[:, 1, :], mul=inv_nm1)
        nc.scalar.dma_start(out=cov2[64:128, s, :], in_=odd)

    # Joint FC for all samples: out[s, o] accumulates over 32 K=128
    # contraction tiles; M=BPC, N=OUT, fp16.
    po = psum_fc.tile([BPC, OUT], F32)
    for u in range(32):
        nc.tensor.matmul(
            po,
            lhsT=cov2[:, :, u],
            rhs=wt_sb[:, u, :],
            start=(u == 0),
            stop=(u == 31),
        )
    o_sb = smalls.tile([BPC, OUT], F32)
    nc.vector.tensor_add(o_sb, po, b4_sb)
    sq = smalls.tile([BPC, OUT], F32)
    nc.vector.tensor_mul(sq, o_sb, o_sb)
    ss = smalls.tile([BPC, 1], F32)
    nc.vector.reduce_sum(out=ss, in_=sq, axis=mybir.AxisListType.X)
    nrm = smalls.tile([BPC, 1], F32)
    nc.scalar.sqrt(nrm, ss)
    nc.vector.tensor_scalar_max(nrm, nrm, 1e-12)
    rn = smalls.tile([BPC, 1], F32)
    nc.vector.reciprocal(rn, nrm)
    nc.vector.tensor_scalar_mul(o_sb, o_sb, rn)
    nc.sync.dma_start(out=out, in_=o_sb)


def build(n_true: int = N_FULL, enable_asserts: bool = False):
    nc = bacc.Bacc(
        "TRN2",
        target_bir_lowering=False,
        debug=False,
        enable_asserts=enable_asserts,
        num_devices=NCORES,
    )
    xs = nc.dram_tensor("xs", [BPC, N_ROWS, DA], F32, kind="ExternalInput").ap()
    wt = nc.dram_tensor("wt", [128, 32 * OUT], FC_DT, kind="ExternalInput").ap()
    b4 = nc.dram_tensor("b4", [BPC, OUT], F32, kind="ExternalInput").ap()
    out = nc.dram_tensor("out", [BPC, OUT], F32, kind="ExternalOutput").ap()
    with tile.TileContext(nc) as tc:
        _cov_kernel(tc, out, xs, wt, b4, n_true)
    nc.compile()
    return nc


_cache: dict = {}


def make_in_maps(x: np.ndarray, W: np.ndarray, b: np.ndarray):
    # Append the ones column and zero-pad rows to whole 128-row tiles on
    # the host (zero rows contribute nothing to S or s).
    bb, nn, _ = x.shape
    xa = np.zeros((bb, N_ROWS, DA), dtype=np.float32)
    xa[:, :nn, :D] = x
    xa[:, :nn, D] = 1.0
    # W^T [4096, 256] -> [p=e+64w, u, o] with t = 2u+w, flattened to
    # [128, 32*256] so the SBUF load is one contiguous 16KiB descriptor
    # per partition.
    wt = np.ascontiguousarray(
        W.T.astype(np.float16)
        .reshape(32, 2, 64, OUT)
        .transpose(1, 2, 0, 3)
        .reshape(128, -1)
    )
    b4 = np.ascontiguousarray(
        np.broadcast_to(np.asarray(b, dtype=np.float32), (BPC, OUT))
    )
    return [
        {
            "xs": np.ascontiguousarray(xa[k * BPC : (k + 1) * BPC]),
            "wt": wt,
            "b4": b4,
        }
        for k in range(NCORES)
    ]


def kernel(x: np.ndarray, W: np.ndarray, b: np.ndarray, **run_kwargs) -> np.ndarray:
    x = np.asarray(x, dtype=np.float32)
    assert x.shape == (B, N_FULL, D), x.shape
    if "nc" not in _cache:
        _cache["nc"] = build(N_FULL)
    nc = _cache["nc"]
    res = bass_utils.run_bass_kernel_spmd(
        nc, make_in_maps(x, W, b), core_ids=list(range(NCORES)), **run_kwargs
    )
    out = np.concatenate([r["out"] for r in res.results], axis=0)
    _cache["last_results"] = res
    return out
